# revision 1
# baseline (speedup 1.0000x reference)
"""GCN layer (GCNConv + relu + dense + relu) on 8 Trainium2 NeuronCores.

Strategy
--------
Math: out = relu(relu(GCNConv(x)) @ W_dense + b_dense) with
GCNConv(x)[v] = dinv[v] * sum_{e: src->v} dinv[src] * (x W_gcn)[src] + b_gcn
(self-loops included as ordinary edges; dinv = rsqrt(indegree incl. self).)

Device plan (2 SPMD launches over 8 cores, nodes sharded 12500/core):
  Launch A: per core, g = dinv_row * (x @ W_gcn), padded to [12800, 64] f32
            rows (tile-major). Host concatenates shards -> table [100000, 64].
  Launch B: per core, edge phase: for each (source-chunk-bin, batch):
            dma_gather rows of the table -> SBUF, dma_scatter_add into a
            private HBM accumulator indexed by local dest row.  All per-edge
            arithmetic is pre-folded into the table (dinv[src]) and the
            epilogue (dinv[dst]).  Epilogue: acc * dinv, +b_gcn, relu,
            @W_dense, +b_dense, relu via PE-transpose packed [128,128] tiles.

Edges are partitioned on host by destination owner core, binned by source
chunk (25000 nodes per chunk so gather indices fit int16), and padded with
dummy edges (src offset 0 -> junk dest row) so every batch is fully valid
and the instruction stream is identical across cores.
"""

import sys

if "/opt/trn_rl_repo" not in sys.path:
    sys.path.insert(0, "/opt/trn_rl_repo")

from dataclasses import dataclass, field

import numpy as np

import concourse.bacc as bacc
import concourse.mybir as mybir
from concourse import tile
from concourse.bass_utils import run_bass_kernel_spmd


@dataclass
class Cfg:
    n_cores: int = 8
    tiles: int = 100          # 128-row tiles per core (multiple of 4)
    in_dim: int = 128
    net_dim: int = 32
    padf: int = 64            # padded feature width of the gather table
    n_chunks: int = 4         # source chunks (gather idx must fit int16)
    # Static per-(core,bin) level capacities: level k holds the k-th edge of
    # each destination, so destinations are unique within a level (required:
    # dma_scatter_add loses concurrent updates to the same destination row
    # within one instruction).  Sized for Binomial(E_bin, 1/nloc) tails.
    level_caps: tuple = (
        (12544, 12544, 12416, 12160, 11520, 10496, 8960, 7296, 5504, 3968,
         2688, 1664, 1024, 640, 384, 256) + (128,) * 12
    )
    seg_max: int = 1024       # max idxs per instruction (HW ring limit: 2048 fails)

    @property
    def nloc(self):
        return (self.tiles * 128 * 125) // 128  # = tiles * 125

    @property
    def npad(self):
        return self.tiles * 128

    @property
    def n(self):
        return self.nloc * self.n_cores

    @property
    def chunk(self):
        return -(-self.n // self.n_chunks)  # ceil

    @property
    def bin_cap(self):
        return sum(self.level_caps)

    @property
    def segments(self):
        """(bin, slot_offset_in_bin, size) per gather/scatter instruction."""
        segs = []
        for b in range(self.n_chunks):
            off = 0
            for cap in self.level_caps:
                k = 0
                while k < cap:
                    sz = min(self.seg_max, cap - k)
                    segs.append((b, off + k, sz))
                    k += sz
                off += cap
        return segs


FULL = Cfg()
assert FULL.n == 100000 and FULL.nloc == 12500 and FULL.chunk == 25000


def _rank_within(lvl, n):
    """rank of each element among equal values of lvl, in array order.

    lvl is the level array in dest-sorted edge order; within a level the
    edges appear in ascending-destination order, so rank = occurrence count
    of that level value so far."""
    ranks = np.zeros(n, dtype=np.int64)
    counters = {}
    # vectorized: stable argsort by lvl groups equal levels preserving order
    order = np.argsort(lvl, kind="stable")
    sl = lvl[order]
    uq, st, ct = np.unique(sl, return_index=True, return_counts=True)
    r = np.arange(n) - np.repeat(st, ct)
    ranks[order] = r
    return ranks


def _f32(x):
    return np.ascontiguousarray(x, dtype=np.float32)


def wrap16(a):
    """Index array [n] -> [128, n//16] int16 layout dma_gather expects."""
    assert a.size % 16 == 0
    w = a.reshape(-1, 16).T
    return np.ascontiguousarray(np.tile(w, (8, 1)), dtype=np.int16)


# ---------------------------------------------------------------- launch A


def build_launch_a(cfg: Cfg):
    nc = bacc.Bacc(
        "TRN2", target_bir_lowering=False, debug=False, num_devices=cfg.n_cores
    )
    T, K, F, PF = cfg.tiles, cfg.in_dim, cfg.net_dim, cfg.padf
    x_d = nc.dram_tensor("x", [cfg.npad, K], mybir.dt.float32, kind="ExternalInput")
    w_d = nc.dram_tensor("w", [K, F], mybir.dt.float32, kind="ExternalInput")
    deg_d = nc.dram_tensor("deg", [128, T], mybir.dt.int32, kind="ExternalInput")
    eye_d = nc.dram_tensor("eye", [128, 128], mybir.dt.float32, kind="ExternalInput")
    g_d = nc.dram_tensor("g", [cfg.npad, PF], mybir.dt.float32, kind="ExternalOutput")

    with tile.TileContext(nc) as tc:
        with (
            tc.tile_pool(name="const", bufs=1) as cpool,
            tc.tile_pool(name="xin", bufs=3) as xpool,
            tc.tile_pool(name="gout", bufs=3) as gpool,
            tc.tile_pool(name="pt", bufs=2, space="PSUM") as ptp,
            tc.tile_pool(name="ph", bufs=2, space="PSUM") as php,
        ):
            w_t = cpool.tile([K, F], mybir.dt.float32)
            eye_t = cpool.tile([128, 128], mybir.dt.float32)
            nc.sync.dma_start(out=w_t[:], in_=w_d[:])
            nc.sync.dma_start(out=eye_t[:], in_=eye_d[:])

            dinv_t = _emit_dinv(nc, cpool, deg_d, T)

            for t in range(T):
                x_t = xpool.tile([128, K], mybir.dt.float32, tag="x")
                nc.sync.dma_start(out=x_t[:], in_=x_d[t * 128 : (t + 1) * 128, :])
                xT_p = ptp.tile([128, 128], mybir.dt.float32, tag="xT")
                nc.tensor.transpose(xT_p[:], x_t[:], eye_t[:])
                xT_t = xpool.tile([128, 128], mybir.dt.float32, tag="xTs")
                nc.vector.tensor_copy(out=xT_t[:], in_=xT_p[:])
                h_p = php.tile([128, F], mybir.dt.float32, tag="h")
                nc.tensor.matmul(h_p[:], xT_t[:], w_t[:], start=True, stop=True)
                g_t = gpool.tile([128, PF], mybir.dt.float32, tag="g")
                nc.vector.memset(g_t[:, F:], 0.0)
                nc.scalar.activation(
                    g_t[:, 0:F],
                    h_p[:],
                    mybir.ActivationFunctionType.Copy,
                    scale=dinv_t[:, t : t + 1],
                )
                nc.sync.dma_start(out=g_d[t * 128 : (t + 1) * 128, :], in_=g_t[:])
    nc.compile()
    return nc


def _emit_dinv(nc, pool, deg_d, T):
    """deg (int32 [128, T]) -> dinv = 1/sqrt(deg) with a Newton step."""
    deg_t = pool.tile([128, T], mybir.dt.int32, name="deg_i")
    degf_t = pool.tile([128, T], mybir.dt.float32, name="deg_f")
    r_t = pool.tile([128, T], mybir.dt.float32, name="recip")
    s_t = pool.tile([128, T], mybir.dt.float32, name="scratch")
    dinv_t = pool.tile([128, T], mybir.dt.float32, name="dinv")
    nc.sync.dma_start(out=deg_t[:], in_=deg_d[:])
    nc.vector.tensor_copy(out=degf_t[:], in_=deg_t[:])
    nc.vector.reciprocal(out=r_t[:], in_=degf_t[:])
    # Newton: r <- r * (2 - d * r) computed as -(r * (d*r - 2))
    nc.vector.tensor_tensor(out=s_t[:], in0=degf_t[:], in1=r_t[:], op=mybir.AluOpType.mult)
    nc.vector.tensor_scalar_add(out=s_t[:], in0=s_t[:], scalar1=-2.0)
    nc.vector.tensor_tensor(out=s_t[:], in0=s_t[:], in1=r_t[:], op=mybir.AluOpType.mult)
    nc.vector.tensor_scalar_mul(out=s_t[:], in0=s_t[:], scalar1=-1.0)
    nc.scalar.sqrt(dinv_t[:], s_t[:])
    return dinv_t


# ---------------------------------------------------------------- launch B


def build_launch_b(cfg: Cfg):
    nc = bacc.Bacc(
        "TRN2", target_bir_lowering=False, debug=False, num_devices=cfg.n_cores
    )
    T, F, PF = cfg.tiles, cfg.net_dim, cfg.padf
    segs = cfg.segments
    NB = len(segs)
    BMAX = max(sz for _, _, sz in segs)

    g_d = nc.dram_tensor("g", [cfg.n, PF], mybir.dt.float32, kind="ExternalInput")
    src_d = nc.dram_tensor(
        "src_i", [NB, 128, BMAX // 16], mybir.dt.int16, kind="ExternalInput"
    )
    dst_d = nc.dram_tensor(
        "dst_i", [NB, 128, BMAX // 16], mybir.dt.int16, kind="ExternalInput"
    )
    gown_d = nc.dram_tensor(
        "gown", [cfg.npad, PF], mybir.dt.float32, kind="ExternalInput"
    )
    deg_d = nc.dram_tensor("deg", [128, T], mybir.dt.int32, kind="ExternalInput")
    bg_d = nc.dram_tensor("bg", [F, 1], mybir.dt.float32, kind="ExternalInput")
    wd_d = nc.dram_tensor("wd", [F, F], mybir.dt.float32, kind="ExternalInput")
    bd_d = nc.dram_tensor("bd", [F, 1], mybir.dt.float32, kind="ExternalInput")
    eye_d = nc.dram_tensor("eye", [128, 128], mybir.dt.float32, kind="ExternalInput")
    out_d = nc.dram_tensor(
        "out", [cfg.nloc, F], mybir.dt.float32, kind="ExternalOutput"
    )
    acc_d = nc.dram_tensor("acc", [cfg.npad, PF], mybir.dt.float32)

    with tile.TileContext(nc) as tc:
        with (
            tc.tile_pool(name="const", bufs=1) as cpool,
            tc.tile_pool(name="idx", bufs=3) as ipool,
            tc.tile_pool(name="msg", bufs=2) as mpool,
            tc.tile_pool(name="epi", bufs=1) as epool,
            tc.tile_pool(name="pt", bufs=2, space="PSUM") as ptp,
            tc.tile_pool(name="ph", bufs=2, space="PSUM") as php,
        ):
            # ---- zero the accumulator
            zero_t = epool.tile(
                [128, cfg.npad * PF // 128], mybir.dt.float32, tag="big"
            )
            nc.vector.memset(zero_t[:], 0.0)
            nc.sync.dma_start(
                out=acc_d.ap().flatten().rearrange("(p f) -> p f", p=128),
                in_=zero_t[:],
            )

            # ---- edge phase: one gather + one scatter per segment; each
            # segment's destinations are unique (level construction), so the
            # scatter-add is race-free; consecutive scatters serialize on acc.
            for si, (cbin, _off, size) in enumerate(segs):
                src_t = ipool.tile([128, BMAX // 16], mybir.dt.int16, tag="si")
                dst_t = ipool.tile([128, BMAX // 16], mybir.dt.int16, tag="di")
                nc.sync.dma_start(out=src_t[:, : size // 16], in_=src_d[si, :, : size // 16])
                nc.sync.dma_start(out=dst_t[:, : size // 16], in_=dst_d[si, :, : size // 16])
                msg_t = mpool.tile([128, BMAX // 128, PF], mybir.dt.float32, tag="m")
                lo = cbin * cfg.chunk
                hi = min(lo + cfg.chunk, cfg.n)
                nc.gpsimd.dma_gather(
                    msg_t[:, : size // 128, :],
                    g_d[lo:hi, :],
                    src_t[:, : size // 16],
                    size,
                    size,
                    PF,
                )
                nc.gpsimd.dma_scatter_add(
                    acc_d[:],
                    msg_t[:, : size // 128, :],
                    dst_t[:, : size // 16],
                    size,
                    size,
                    PF,
                )

            # ---- epilogue
            eye_t = cpool.tile([128, 128], mybir.dt.float32)
            nc.sync.dma_start(out=eye_t[:], in_=eye_d[:])
            wpack_t = cpool.tile([128, 128], mybir.dt.float32)
            nc.vector.memset(wpack_t[:], 0.0)
            bg_t = cpool.tile([128, 1], mybir.dt.float32)
            bd_t = cpool.tile([128, 1], mybir.dt.float32)
            for grp in range(4):
                sl = slice(F * grp, F * grp + F)
                nc.sync.dma_start(out=wpack_t[sl, sl], in_=wd_d[:])
                nc.sync.dma_start(out=bg_t[sl, :], in_=bg_d[:])
                nc.sync.dma_start(out=bd_t[sl, :], in_=bd_d[:])
            dinv_t = _emit_dinv(nc, cpool, deg_d, T)

            # acc, p-major: partition p holds rows [p*T, (p+1)*T)
            acc_t = epool.tile([128, T, PF], mybir.dt.float32, tag="big")
            nc.sync.dma_start(
                out=acc_t[:].rearrange("p t f -> p (t f)"),
                in_=acc_d.ap().flatten().rearrange("(p f) -> p f", p=128),
            )
            gown_t = epool.tile([128, T, PF], mybir.dt.float32)
            nc.sync.dma_start(
                out=gown_t[:].rearrange("p t f -> p (t f)"),
                in_=gown_d.ap().flatten().rearrange("(p f) -> p f", p=128),
            )
            h1_t = epool.tile([128, T, F], mybir.dt.float32)
            nc.vector.tensor_tensor(
                out=h1_t[:],
                in0=acc_t[:, :, 0:F],
                in1=gown_t[:, :, 0:F],
                op=mybir.AluOpType.add,
            )
            nc.vector.tensor_tensor(
                out=h1_t[:],
                in0=h1_t[:],
                in1=dinv_t[:].unsqueeze(2).broadcast_to((128, T, F)),
                op=mybir.AluOpType.mult,
            )
            out_t = epool.tile([128, T, F], mybir.dt.float32)
            for j in range(T // 4):
                h1T_p = ptp.tile([128, 128], mybir.dt.float32, tag="h1T")
                nc.tensor.transpose(
                    h1T_p[:],
                    h1_t[:, 4 * j : 4 * j + 4, :].rearrange("p a b -> p (a b)"),
                    eye_t[:],
                )
                h1T_t = mpool.tile([128, 128], mybir.dt.float32, tag="h1Ts")
                nc.scalar.activation(
                    h1T_t[:], h1T_p[:], mybir.ActivationFunctionType.Relu, bias=bg_t[:]
                )
                h2T_p = php.tile([128, 128], mybir.dt.float32, tag="h2T")
                nc.tensor.matmul(h2T_p[:], wpack_t[:], h1T_t[:], start=True, stop=True)
                h2T_t = mpool.tile([128, 128], mybir.dt.float32, tag="h2Ts")
                nc.scalar.activation(
                    h2T_t[:], h2T_p[:], mybir.ActivationFunctionType.Relu, bias=bd_t[:]
                )
                o_p = ptp.tile([128, 128], mybir.dt.float32, tag="oT")
                nc.tensor.transpose(o_p[:], h2T_t[:], eye_t[:])
                nc.vector.tensor_copy(
                    out=out_t[:, 4 * j : 4 * j + 4, :].rearrange("p a b -> p (a b)"),
                    in_=o_p[:],
                )
            nc.sync.dma_start(
                out=out_d[:],
                in_=out_t[0:125, :, :],
            )
    nc.compile()
    return nc


# ---------------------------------------------------------------- host side


def host_prep(x, edge_index, W_gcn, b_gcn, W_dense, b_dense, cfg: Cfg):
    n, nloc = cfg.n, cfg.nloc
    row = np.asarray(edge_index[0])
    col = np.asarray(edge_index[1])
    deg = np.bincount(col, minlength=n).astype(np.int64) + 1  # + self-loop

    eye = np.eye(128, dtype=np.float32)
    W_gcn = _f32(W_gcn)
    b_gcn = _f32(b_gcn).reshape(cfg.net_dim, 1)
    W_dense = _f32(W_dense)
    b_dense = _f32(b_dense).reshape(cfg.net_dim, 1)
    x = _f32(x)

    owner = col // nloc
    in_a, in_b = [], []
    for c in range(cfg.n_cores):
        m = owner == c
        srcs = row[m]
        dstl = (col[m] - c * nloc).astype(np.int64)

        # deg layouts
        dloc = deg[c * nloc : (c + 1) * nloc]
        dpad = np.ones(cfg.npad, dtype=np.int32)
        dpad[:nloc] = dloc
        deg_a = dpad.reshape(cfg.tiles, 128).T.copy()  # [128, T] tile-major
        deg_b = dpad.reshape(128, cfg.tiles).copy()    # [128, T] p-major

        xpad = np.zeros((cfg.npad, cfg.in_dim), dtype=np.float32)
        xpad[:nloc] = x[c * nloc : (c + 1) * nloc]
        in_a.append({"x": xpad, "w": W_gcn, "deg": deg_a, "eye": eye})

        # bin edges by source chunk; within a bin, assign each edge a level
        # (its ordinal among edges sharing the destination) and lay levels
        # out contiguously with static capacities.
        cbin = srcs // cfg.chunk
        segs = cfg.segments
        bm = max(sz for _, _, sz in segs)
        src_i = np.zeros((len(segs), 128, bm // 16), dtype=np.int16)
        dst_i = np.zeros((len(segs), 128, bm // 16), dtype=np.int16)
        caps = np.array(cfg.level_caps)
        cap_off = np.concatenate([[0], np.cumsum(caps)])
        for k in range(cfg.n_chunks):
            mk = cbin == k
            so = (srcs[mk] - k * cfg.chunk).astype(np.int64)
            dl = dstl[mk]
            # level of each edge: ordinal within its destination group
            order = np.argsort(dl, kind="stable")
            ds, ss = dl[order], so[order]
            uniq, starts, counts = np.unique(
                ds, return_index=True, return_counts=True
            )
            lvl = np.arange(ds.size) - np.repeat(starts, counts)
            assert counts.max() <= len(caps), (c, k, counts.max(), len(caps))
            lc = np.bincount(lvl, minlength=len(caps))
            assert (lc <= caps).all(), (c, k, lc[lc > caps], caps[lc > caps])
            # slot index: level base + rank within level (dest-ascending)
            slot = cap_off[lvl] + _rank_within(lvl, ds.size)
            bin_src = np.zeros(cfg.bin_cap, dtype=np.int64)
            bin_dst = np.full(cfg.bin_cap, cfg.npad - 1, dtype=np.int64)
            bin_src[slot] = ss
            bin_dst[slot] = ds
            for si, (b2, off, sz) in enumerate(segs):
                if b2 != k:
                    continue
                src_i[si, :, : sz // 16] = wrap16(bin_src[off : off + sz])
                dst_i[si, :, : sz // 16] = wrap16(bin_dst[off : off + sz])
        in_b.append(
            {
                "src_i": src_i,
                "dst_i": dst_i,
                "deg": deg_b,
                "bg": b_gcn,
                "wd": W_dense,
                "bd": b_dense,
                "eye": eye,
            }
        )
    return in_a, in_b


def assemble_table(res_a, cfg: Cfg):
    return np.concatenate(
        [res_a[c]["g"][: cfg.nloc] for c in range(cfg.n_cores)], axis=0
    )


def assemble_out(res_b, cfg: Cfg):
    return np.concatenate(
        [res_b[c]["out"] for c in range(cfg.n_cores)], axis=0
    )


_NC_CACHE = {}


def _get_ncs(cfg: Cfg):
    key = (cfg.n, cfg.tiles)
    if key not in _NC_CACHE:
        _NC_CACHE[key] = (build_launch_a(cfg), build_launch_b(cfg))
    return _NC_CACHE[key]


def _add_table(in_b, table, cfg: Cfg):
    for c, m in enumerate(in_b):
        m["g"] = table
        gown = np.zeros((cfg.npad, cfg.padf), dtype=np.float32)
        gown[: cfg.nloc] = table[c * cfg.nloc : (c + 1) * cfg.nloc]
        m["gown"] = gown  # epilogue reads it p-major via a flat AP


def kernel(x, edge_index, W_gcn, b_gcn, W_dense, b_dense):
    cfg = FULL
    nc_a, nc_b = _get_ncs(cfg)
    in_a, in_b = host_prep(x, edge_index, W_gcn, b_gcn, W_dense, b_dense, cfg)
    core_ids = list(range(cfg.n_cores))
    res_a = run_bass_kernel_spmd(nc_a, in_a, core_ids).results
    table = assemble_table(res_a, cfg)
    _add_table(in_b, table, cfg)
    res_b = run_bass_kernel_spmd(nc_b, in_b, core_ids).results
    return assemble_out(res_b, cfg)



# revision 11
# speedup vs baseline: 4.8675x; 4.8675x over previous
"""GCN layer (GCNConv + relu + dense + relu) on 8 Trainium2 NeuronCores.

Strategy (v2 — PE segment-sum, no dma_scatter_add)
--------------------------------------------------
Math: out = relu(relu(GCNConv(x)) @ W_dense + b_dense) with
GCNConv(x)[v] = dinv[v] * sum_{e: src->v} dinv[src] * (x W_gcn)[src] + b_gcn
(self-loops included as ordinary edges; dinv = rsqrt(indegree incl. self).)

Device plan (2 SPMD launches over 8 cores, nodes sharded 12500/core):
  Launch A: per core, g = dinv_row * (x @ W_gcn) as bf16 rows padded to
            128 cols (256B — the dma_gather minimum granule).  Host
            pre-transposes x so each tile is one matmul (no PE transpose).
            Host concatenates shards -> table [100000, 128] bf16.
  Launch B: per core, edges sorted by (dst-tile-group, src-chunk, dst-tile).
            Per gather instruction (<=15 batches of 128 edges):
            dma_gather rows of the table -> SBUF; DVE builds a one-hot
            [128e x 128d] from dst offsets via is_equal vs a host-sent
            iota; PE matmul lhsT=msgs[:, 0:32] rhs=onehot accumulates
            feature-major segment sums in PSUM (fp32, exact).  Epilogue is
            feature-major: dinv columns via DVE, biases per-partition via
            ACT, dense layer is one matmul per 4-tile group.  Output is
            written feature-major [32, 12544]; host transposes (free).

No gpsimd per-edge descriptor generation for the scatter side at all, and
the gather side uses 1920-index instructions (ring limit 128 descs =
n/16+1) spread over 4 SWDGE queues.
"""

import sys

if "/opt/trn_rl_repo" not in sys.path:
    sys.path.insert(0, "/opt/trn_rl_repo")

from dataclasses import dataclass

import numpy as np

import concourse.bacc as bacc
import concourse.mybir as mybir
from concourse import tile
from concourse.bass_utils import run_bass_kernel_spmd


@dataclass(frozen=True)
class Cfg:
    n_cores: int = 8
    nloc: int = 12500
    ntiles: int = 98              # 128-row dst tiles per core (12544 padded)
    in_dim: int = 128
    net_dim: int = 32
    row: int = 128                # table row width (bf16) = 256B
    n_chunks: int = 4             # src chunks of 25000 (int16 gather idx)
    chunk: int = 25000
    tg: int = 4                   # dst tiles per PSUM/epilogue group
    gmax: int = 8                 # max 128-edge batches per dma_gather (1024,
                                  # the Q7 idx-scratch hard limit)

    @property
    def npad(self):
        return self.ntiles * 128  # 12544

    @property
    def n(self):
        return self.nloc * self.n_cores


FULL = Cfg()
assert FULL.n == 100000 and FULL.chunk * FULL.n_chunks == FULL.n


def _f32(x):
    return np.ascontiguousarray(x, dtype=np.float32)


def wrap16(a):
    """Index array [n] -> [128, n//16] int16 layout dma_gather expects."""
    assert a.size % 16 == 0
    w = a.reshape(-1, 16).T
    return np.ascontiguousarray(np.tile(w, (8, 1)), dtype=np.int16)


def _emit_dinv(nc, pool, deg_d, p, w, name):
    """deg (int32 [p, w]) -> dinv = 1/sqrt(deg) with a Newton step."""
    deg_t = pool.tile([p, w], mybir.dt.int32, name=f"{name}_i", tag="dinv_i")
    degf_t = pool.tile([p, w], mybir.dt.float32, name=f"{name}_f", tag="dinv_f")
    r_t = pool.tile([p, w], mybir.dt.float32, name=f"{name}_r", tag="dinv_r")
    s_t = pool.tile([p, w], mybir.dt.float32, name=f"{name}_s", tag="dinv_s")
    dinv_t = pool.tile([p, w], mybir.dt.float32, name=f"{name}_v", tag="dinv_v")
    nc.sync.dma_start(out=deg_t[:], in_=deg_d[:])
    nc.vector.tensor_copy(out=degf_t[:], in_=deg_t[:])
    nc.vector.reciprocal(out=r_t[:], in_=degf_t[:])
    # Newton: r <- r * (2 - d * r) computed as -(r * (d*r - 2))
    nc.vector.tensor_tensor(out=s_t[:], in0=degf_t[:], in1=r_t[:], op=mybir.AluOpType.mult)
    nc.vector.tensor_scalar_add(out=s_t[:], in0=s_t[:], scalar1=-2.0)
    nc.vector.tensor_tensor(out=s_t[:], in0=s_t[:], in1=r_t[:], op=mybir.AluOpType.mult)
    nc.vector.tensor_scalar_mul(out=s_t[:], in0=s_t[:], scalar1=-1.0)
    nc.scalar.sqrt(dinv_t[:], s_t[:])
    return dinv_t


# ---------------------------------------------------------------- layout


class Layout:
    """Static (core-independent) batch/instruction structure for launch B.

    nb[t, k]   : number of 128-edge batches for (dst tile t, src chunk k)
    batches    : per batch B: (tile_in_tg, start, stop)
    instrs     : per gather instr: (chunk k, B0, nbatches)
    tg_of_instr: tile-group index owning each instruction (epilogue order)
    """

    def __init__(self, cfg: Cfg, nb):
        self.cfg = cfg
        self.nb = nb  # [ntiles, n_chunks]
        T, K, TG = cfg.ntiles, cfg.n_chunks, cfg.tg
        self.instrs = []          # (k, B0, nb, tg_idx)
        self.tg_sizes = []
        self.tk_range = {}        # (t, k) -> (B_start, B_end)
        B = 0
        ntg = (T + TG - 1) // TG
        for g in range(ntg):
            tiles = list(range(g * TG, min((g + 1) * TG, T)))
            self.tg_sizes.append(len(tiles))
            for k in range(K):
                run_b0 = B
                for t in tiles:
                    n = int(nb[t, k])
                    self.tk_range[(t, k)] = (B, B + n)
                    B += n
                run_nb = B - run_b0
                # split the run into instructions of <= gmax batches
                ni = (run_nb + cfg.gmax - 1) // cfg.gmax
                off = run_b0
                for i in range(ni):
                    sz = (run_nb + ni - 1 - i) // ni  # even split
                    self.instrs.append((k, off, sz, g))
                    off += sz
                assert off == B
        self.NB = B
        self.NI = len(self.instrs)
        self.ntg = ntg
        # batch -> (instr index, offset within instr)
        self.batch_pos = [None] * B
        for i, (k, B0, nbt, g) in enumerate(self.instrs):
            for b in range(nbt):
                self.batch_pos[B0 + b] = (i, b)
        # instrs grouped per tg
        self.tg_instrs = [[] for _ in range(ntg)]
        for i, (k, B0, nbt, g) in enumerate(self.instrs):
            self.tg_instrs[g].append(i)


def make_layout(cfg: Cfg, counts):
    """counts: [n_cores, ntiles, n_chunks] edge counts -> static Layout."""
    mx = counts.max(axis=0)
    nb = np.maximum((mx + 127) // 128, 1).astype(np.int64)
    return Layout(cfg, nb)


# ---------------------------------------------------------------- launch A


def build_launch_a(cfg: Cfg):
    nc = bacc.Bacc(
        "TRN2", target_bir_lowering=False, debug=False, num_devices=cfg.n_cores
    )
    T, K, F, R = cfg.ntiles, cfg.in_dim, cfg.net_dim, cfg.row
    xT_d = nc.dram_tensor("xT", [K, cfg.npad], mybir.dt.float32, kind="ExternalInput")
    w_d = nc.dram_tensor("w", [K, F], mybir.dt.float32, kind="ExternalInput")
    deg_d = nc.dram_tensor("deg", [128, T], mybir.dt.int32, kind="ExternalInput")
    g_d = nc.dram_tensor("g", [cfg.npad, R], mybir.dt.bfloat16, kind="ExternalOutput")

    with tile.TileContext(nc) as tc:
        with (
            tc.tile_pool(name="const", bufs=1) as cpool,
            tc.tile_pool(name="xin", bufs=4) as xpool,
            tc.tile_pool(name="gout", bufs=4) as gpool,
            tc.tile_pool(name="ph", bufs=4, space="PSUM") as php,
        ):
            w_t = cpool.tile([K, F], mybir.dt.float32)
            nc.sync.dma_start(out=w_t[:], in_=w_d[:])
            dinv_t = _emit_dinv(nc, cpool, deg_d, 128, T, "dinv")

            for t in range(T):
                xT_t = xpool.tile([128, 128], mybir.dt.float32, tag="x")
                nc.sync.dma_start(out=xT_t[:], in_=xT_d[:, t * 128 : (t + 1) * 128])
                h_p = php.tile([128, F], mybir.dt.float32, tag="h")
                nc.tensor.matmul(h_p[:], xT_t[:], w_t[:], start=True, stop=True)
                g_t = gpool.tile([128, R], mybir.dt.bfloat16, tag="g")
                nc.scalar.activation(
                    g_t[:, 0:F],
                    h_p[:],
                    mybir.ActivationFunctionType.Copy,
                    scale=dinv_t[:, t : t + 1],
                )
                nc.vector.memset(g_t[:, F:R], 0.0)
                nc.sync.dma_start(out=g_d[t * 128 : (t + 1) * 128, :], in_=g_t[:])
    nc.compile()
    return nc


# ---------------------------------------------------------------- launch B


def build_launch_b(cfg: Cfg, lay: Layout):
    nc = bacc.Bacc(
        "TRN2",
        target_bir_lowering=False,
        debug=False,
        num_devices=cfg.n_cores,
        num_swdge_queues=4,
    )
    F, R, TG = cfg.net_dim, cfg.row, cfg.tg
    NB, NI = lay.NB, lay.NI
    GM = cfg.gmax

    g_d = nc.dram_tensor("g", [cfg.n, R], mybir.dt.bfloat16, kind="ExternalInput")
    src_d = nc.dram_tensor(
        "src_i", [NI, 128, GM * 8], mybir.dt.int16, kind="ExternalInput"
    )
    dstv_d = nc.dram_tensor("dstv", [128, NB], mybir.dt.bfloat16, kind="ExternalInput")
    iota_d = nc.dram_tensor("iota", [128, 128], mybir.dt.bfloat16, kind="ExternalInput")
    degfm_d = nc.dram_tensor("degfm", [F, cfg.npad], mybir.dt.int32, kind="ExternalInput")
    bg_d = nc.dram_tensor("bg", [F, 1], mybir.dt.float32, kind="ExternalInput")
    wd_d = nc.dram_tensor("wd", [F, F], mybir.dt.float32, kind="ExternalInput")
    bd_d = nc.dram_tensor("bd", [F, 1], mybir.dt.float32, kind="ExternalInput")
    out_d = nc.dram_tensor("out", [F, cfg.npad], mybir.dt.float32, kind="ExternalOutput")

    with tile.TileContext(nc) as tc:
        with (
            tc.tile_pool(name="const", bufs=1) as cpool,
            tc.tile_pool(name="idx", bufs=14) as ipool,
            tc.tile_pool(name="msg", bufs=14) as mpool,
            tc.tile_pool(name="oh", bufs=14) as opool,
            tc.tile_pool(name="epi", bufs=2) as epool,
            tc.tile_pool(name="acc", bufs=4, space="PSUM") as apool,
            tc.tile_pool(name="h2", bufs=2, space="PSUM") as hpool,
        ):
            iota_t = cpool.tile([128, 128], mybir.dt.bfloat16)
            nc.sync.dma_start(out=iota_t[:], in_=iota_d[:])
            dstv_t = cpool.tile([128, NB], mybir.dt.bfloat16)
            nc.sync.dma_start(out=dstv_t[:], in_=dstv_d[:])
            bg_t = cpool.tile([F, 1], mybir.dt.float32)
            nc.sync.dma_start(out=bg_t[:], in_=bg_d[:])
            wd_t = cpool.tile([F, F], mybir.dt.float32)
            nc.sync.dma_start(out=wd_t[:], in_=wd_d[:])
            bd_t = cpool.tile([F, 1], mybir.dt.float32)
            nc.sync.dma_start(out=bd_t[:], in_=bd_d[:])
            # dinv feature-major [32, npad], computed in column chunks to
            # bound the temporary SBUF footprint (each temp is npad/4 cols).
            dinv_t = cpool.tile([F, cfg.npad], mybir.dt.float32, name="dinvfm")
            CH = cfg.npad // 4
            with tc.tile_pool(name="dtmp", bufs=1) as dpool:
                for j in range(4):
                    sl = slice(j * CH, (j + 1) * CH)
                    dj = _emit_dinv(nc, dpool, degfm_d[:, sl], F, CH, f"dv{j}")
                    nc.vector.tensor_copy(out=dinv_t[:, sl], in_=dj[:])

            for g in range(lay.ntg):
                tn = lay.tg_sizes[g]
                W = tn * 128
                c0 = g * TG * 128
                # ---- gathers + one-hots for all of this group's instrs
                tiles_of = {}
                for i in lay.tg_instrs[g]:
                    k, B0, nbt, _g = lay.instrs[i]
                    nidx = nbt * 128
                    idx_t = ipool.tile([128, GM * 8], mybir.dt.int16, tag="idx")
                    nc.sync.dma_start(
                        out=idx_t[:, : nbt * 8], in_=src_d[i, :, : nbt * 8]
                    )
                    msg_t = mpool.tile([128, GM, R], mybir.dt.bfloat16, tag="m")
                    nc.gpsimd.dma_gather(
                        msg_t[:, :nbt, :],
                        g_d[k * cfg.chunk : (k + 1) * cfg.chunk, :],
                        idx_t[:, : nbt * 8],
                        nidx,
                        nidx,
                        R,
                        queue_num=i % 4,
                    )
                    oh_t = opool.tile([128, GM, 128], mybir.dt.bfloat16, tag="oh")
                    nc.vector.tensor_tensor(
                        out=oh_t[:, :nbt, :],
                        in0=iota_t[:].unsqueeze(1).broadcast_to((128, nbt, 128)),
                        in1=dstv_t[:, B0 : B0 + nbt]
                        .unsqueeze(2)
                        .broadcast_to((128, nbt, 128)),
                        op=mybir.AluOpType.is_equal,
                    )
                    tiles_of[i] = (msg_t, oh_t)

                # ---- tile-major accumulation: one open PSUM group at a time,
                # each accumulator owns a full bank (start=True resets the
                # bank, so groups must not interleave within one).
                h1_t = epool.tile([F, TG * 128], mybir.dt.float32, tag="h1")
                for tl in range(tn):
                    t = g * TG + tl
                    acc_t = apool.tile([128, 512], mybir.dt.float32, tag="acc")
                    batches = []
                    for k in range(cfg.n_chunks):
                        b0, b1 = lay.tk_range[(t, k)]
                        batches.extend(range(b0, b1))
                    for j, B in enumerate(batches):
                        i, b = lay.batch_pos[B]
                        msg_t, oh_t = tiles_of[i]
                        nc.tensor.matmul(
                            acc_t[0:F, 0:128],
                            msg_t[:, b, 0:F],
                            oh_t[:, b, :],
                            start=(j == 0),
                            stop=(j == len(batches) - 1),
                        )
                    nc.vector.tensor_tensor(
                        out=h1_t[:, tl * 128 : (tl + 1) * 128],
                        in0=acc_t[0:F, 0:128],
                        in1=dinv_t[:, (g * TG + tl) * 128 : (g * TG + tl + 1) * 128],
                        op=mybir.AluOpType.mult,
                    )

                # ---- epilogue for the group
                r1_t = epool.tile([F, TG * 128], mybir.dt.float32, tag="r1")
                nc.scalar.activation(
                    r1_t[:, :W], h1_t[:, :W],
                    mybir.ActivationFunctionType.Relu, bias=bg_t[:],
                )
                h2_p = hpool.tile([F, 512], mybir.dt.float32, tag="h2")
                nc.tensor.matmul(
                    h2_p[:, :W], wd_t[:], r1_t[:, :W], start=True, stop=True
                )
                o_t = epool.tile([F, TG * 128], mybir.dt.float32, tag="o")
                nc.scalar.activation(
                    o_t[:, :W], h2_p[:, :W],
                    mybir.ActivationFunctionType.Relu, bias=bd_t[:],
                )
                nc.sync.dma_start(out=out_d[:, c0 : c0 + W], in_=o_t[:, :W])
    nc.compile()
    return nc


# ---------------------------------------------------------------- host side


def host_prep(x, edge_index, W_gcn, b_gcn, W_dense, b_dense, cfg: Cfg):
    n, nloc = cfg.n, cfg.nloc
    row = np.asarray(edge_index[0]).astype(np.int64)
    col = np.asarray(edge_index[1]).astype(np.int64)
    deg = (np.bincount(col, minlength=n) + 1).astype(np.int32)  # + self-loop

    W_gcn = _f32(W_gcn)
    b_gcn = _f32(b_gcn).reshape(cfg.net_dim, 1)
    W_dense = _f32(W_dense)
    b_dense = _f32(b_dense).reshape(cfg.net_dim, 1)
    x = _f32(x)

    import ml_dtypes

    iota = np.tile(np.arange(128), (128, 1)).astype(ml_dtypes.bfloat16)

    # ---- per-core edge sets (dst-sharded) + self loops
    owner = col // nloc
    per_core = []
    counts = np.zeros((cfg.n_cores, cfg.ntiles, cfg.n_chunks), dtype=np.int64)
    for c in range(cfg.n_cores):
        m = owner == c
        srcs = row[m]
        dstl = col[m] - c * nloc
        loop = np.arange(nloc, dtype=np.int64)
        srcs = np.concatenate([srcs, loop + c * nloc])
        dstl = np.concatenate([dstl, loop])
        t = dstl >> 7
        k = srcs // cfg.chunk
        np.add.at(counts[c], (t, k), 1)
        per_core.append((srcs, dstl, t, k))

    lay = make_layout(cfg, counts)

    in_a, in_b = [], []
    for c in range(cfg.n_cores):
        srcs, dstl, t, k = per_core[c]
        # deg layouts
        dpad = np.ones(cfg.npad, dtype=np.int32)
        dpad[:nloc] = deg[c * nloc : (c + 1) * nloc]
        deg_a = np.ascontiguousarray(dpad.reshape(cfg.ntiles, 128).T)  # [128, T]
        deg_fm = np.ascontiguousarray(
            np.tile(dpad, (cfg.net_dim, 1))
        )  # [32, npad]

        xT = np.zeros((cfg.in_dim, cfg.npad), dtype=np.float32)
        xT[:, :nloc] = x[c * nloc : (c + 1) * nloc].T
        in_a.append({"xT": xT, "w": W_gcn, "deg": deg_a})

        # ---- slot assignment in (TG, k, t) order
        src_all = np.zeros(lay.NB * 128, dtype=np.int64)
        dst_all = np.full(lay.NB * 128, -1.0, dtype=np.float64)
        # compute slot base per (t, k) in layout order
        base = np.zeros((cfg.ntiles, cfg.n_chunks), dtype=np.int64)
        B = 0
        for g in range(lay.ntg):
            tiles = range(g * cfg.tg, min((g + 1) * cfg.tg, cfg.ntiles))
            for kk in range(cfg.n_chunks):
                for tt in tiles:
                    base[tt, kk] = B * 128
                    B += int(lay.nb[tt, kk])
        # order edges by (t, k) then place sequentially
        order = np.lexsort((k, t))
        ts, ks = t[order], k[order]
        so = (srcs[order] - ks * cfg.chunk).astype(np.int64)
        do = (dstl[order] & 127).astype(np.int64)
        grp = ts * cfg.n_chunks + ks
        # rank within group
        uq, starts_, cnts_ = np.unique(grp, return_index=True, return_counts=True)
        rank = np.arange(grp.size) - np.repeat(starts_, cnts_)
        slot = base[ts, ks] + rank
        src_all[slot] = so
        dst_all[slot] = do

        src_i = np.zeros((lay.NI, 128, cfg.gmax * 8), dtype=np.int16)
        for i, (kk, B0, nbt, _g) in enumerate(lay.instrs):
            seg = src_all[B0 * 128 : (B0 + nbt) * 128]
            src_i[i, :, : nbt * 8] = wrap16(seg)
        dstv = np.ascontiguousarray(dst_all.reshape(lay.NB, 128).T).astype(
            ml_dtypes.bfloat16
        )

        in_b.append(
            {
                "src_i": src_i,
                "dstv": dstv,
                "iota": iota,
                "degfm": deg_fm,
                "bg": b_gcn,
                "wd": W_dense,
                "bd": b_dense,
            }
        )
    return in_a, in_b, lay


def assemble_table(res_a, cfg: Cfg):
    return np.ascontiguousarray(
        np.concatenate([res_a[c]["g"][: cfg.nloc] for c in range(cfg.n_cores)], axis=0)
    )


def assemble_out(res_b, cfg: Cfg):
    return np.ascontiguousarray(
        np.concatenate(
            [res_b[c]["out"][:, : cfg.nloc].T for c in range(cfg.n_cores)], axis=0
        )
    ).astype(np.float32)


def _add_table(in_b, table, cfg: Cfg):
    for m in in_b:
        m["g"] = table


def kernel(x, edge_index, W_gcn, b_gcn, W_dense, b_dense):
    cfg = FULL
    in_a, in_b, lay = host_prep(x, edge_index, W_gcn, b_gcn, W_dense, b_dense, cfg)
    nc_a = build_launch_a(cfg)
    nc_b = build_launch_b(cfg, lay)
    core_ids = list(range(cfg.n_cores))
    res_a = run_bass_kernel_spmd(nc_a, in_a, core_ids).results
    table = assemble_table(res_a, cfg)
    _add_table(in_b, table, cfg)
    res_b = run_bass_kernel_spmd(nc_b, in_b, core_ids).results
    return assemble_out(res_b, cfg)


# revision 17
# speedup vs baseline: 5.0855x; 1.0448x over previous
"""GCN layer (GCNConv + relu + dense + relu) on 8 Trainium2 NeuronCores.

Strategy (v2 — PE segment-sum, no dma_scatter_add)
--------------------------------------------------
Math: out = relu(relu(GCNConv(x)) @ W_dense + b_dense) with
GCNConv(x)[v] = dinv[v] * sum_{e: src->v} dinv[src] * (x W_gcn)[src] + b_gcn
(self-loops included as ordinary edges; dinv = rsqrt(indegree incl. self).)

Device plan (2 SPMD launches over 8 cores, nodes sharded 12500/core):
  Launch A: per core, g = dinv_row * (x @ W_gcn) as bf16 rows padded to
            128 cols (256B — the dma_gather minimum granule).  Host
            pre-transposes x so each tile is one matmul (no PE transpose).
            Host concatenates shards -> table [100000, 128] bf16.
  Launch B: per core, edges sorted by (dst-tile-group, src-chunk, dst-tile).
            Per gather instruction (<=15 batches of 128 edges):
            dma_gather rows of the table -> SBUF; DVE builds a one-hot
            [128e x 128d] from dst offsets via is_equal vs a host-sent
            iota; PE matmul lhsT=msgs[:, 0:32] rhs=onehot accumulates
            feature-major segment sums in PSUM (fp32, exact).  Epilogue is
            feature-major: dinv columns via DVE, biases per-partition via
            ACT, dense layer is one matmul per 4-tile group.  Output is
            written feature-major [32, 12544]; host transposes (free).

No gpsimd per-edge descriptor generation for the scatter side at all, and
the gather side uses 1920-index instructions (ring limit 128 descs =
n/16+1) spread over 4 SWDGE queues.
"""

import sys

if "/opt/trn_rl_repo" not in sys.path:
    sys.path.insert(0, "/opt/trn_rl_repo")

from dataclasses import dataclass

import numpy as np

import concourse.bacc as bacc
import concourse.mybir as mybir
from concourse import tile
from concourse.bass_utils import run_bass_kernel_spmd


@dataclass(frozen=True)
class Cfg:
    n_cores: int = 8
    nloc: int = 12500
    ntiles: int = 98              # 128-row dst tiles per core (12544 padded)
    in_dim: int = 128
    net_dim: int = 32
    row: int = 128                # table row width (bf16) = 256B
    n_chunks: int = 4             # src chunks of 25000 (int16 gather idx)
    chunk: int = 25000
    tg: int = 4                   # dst tiles per PSUM/epilogue group
    gmax: int = 8                 # max 128-edge batches per dma_gather (1024,
                                  # the Q7 idx-scratch hard limit)

    @property
    def npad(self):
        return self.ntiles * 128  # 12544

    @property
    def n(self):
        return self.nloc * self.n_cores


FULL = Cfg()
assert FULL.n == 100000 and FULL.chunk * FULL.n_chunks == FULL.n


def _f32(x):
    return np.ascontiguousarray(x, dtype=np.float32)


def wrap16(a):
    """Index array [n] -> [128, n//16] int16 layout dma_gather expects."""
    assert a.size % 16 == 0
    w = a.reshape(-1, 16).T
    return np.ascontiguousarray(np.tile(w, (8, 1)), dtype=np.int16)


def _emit_dinv(nc, pool, deg_d, p, w, name):
    """deg (int32 [p, w]) -> dinv = 1/sqrt(deg) with a Newton step."""
    deg_t = pool.tile([p, w], mybir.dt.int32, name=f"{name}_i", tag="dinv_i")
    degf_t = pool.tile([p, w], mybir.dt.float32, name=f"{name}_f", tag="dinv_f")
    r_t = pool.tile([p, w], mybir.dt.float32, name=f"{name}_r", tag="dinv_r")
    s_t = pool.tile([p, w], mybir.dt.float32, name=f"{name}_s", tag="dinv_s")
    dinv_t = pool.tile([p, w], mybir.dt.float32, name=f"{name}_v", tag="dinv_v")
    nc.sync.dma_start(out=deg_t[:], in_=deg_d[:])
    nc.vector.tensor_copy(out=degf_t[:], in_=deg_t[:])
    nc.vector.reciprocal(out=r_t[:], in_=degf_t[:])
    # Newton: r <- r * (2 - d * r) computed as -(r * (d*r - 2))
    nc.vector.tensor_tensor(out=s_t[:], in0=degf_t[:], in1=r_t[:], op=mybir.AluOpType.mult)
    nc.vector.tensor_scalar_add(out=s_t[:], in0=s_t[:], scalar1=-2.0)
    nc.vector.tensor_tensor(out=s_t[:], in0=s_t[:], in1=r_t[:], op=mybir.AluOpType.mult)
    nc.vector.tensor_scalar_mul(out=s_t[:], in0=s_t[:], scalar1=-1.0)
    nc.scalar.sqrt(dinv_t[:], s_t[:])
    return dinv_t


# ---------------------------------------------------------------- layout


class Layout:
    """Static (core-independent) slot/instruction structure for launch B.

    Edge slots are allocated in 16-slot units (the dma_gather index
    granularity) so per-(tile, chunk) padding is 16, not 128.  A gather
    instruction covers <= 64 units (1024 idxs); its output is viewed as
    128-slot batches, and a (tile, chunk) segment maps to a static list of
    (instr, batch, lo, hi) partition-range pieces for the PE matmuls.
    """

    UNIT = 64  # slots per unit; pieces start at 0/64 (legal PE bases)

    def __init__(self, cfg: Cfg, nb16):
        self.cfg = cfg
        self.nb16 = nb16  # [ntiles, n_chunks] units per (t, k)
        U = self.UNIT
        T, K, TG = cfg.ntiles, cfg.n_chunks, cfg.tg
        UMAX = cfg.gmax * 128 // U  # units per instr
        self.tg_sizes = []
        self.tk_units = {}       # (t, k) -> (u0, u1) global unit span
        self.instrs = []         # (k, u0, n_units, g, batch_base)
        self.tg_instrs = []
        ntg = (T + TG - 1) // TG
        u = 0
        B = 0
        instr_of_unit = []
        for g in range(ntg):
            tiles = list(range(g * TG, min((g + 1) * TG, T)))
            self.tg_sizes.append(len(tiles))
            self.tg_instrs.append([])
            for k in range(K):
                run_u0 = u
                for t in tiles:
                    n = int(nb16[t, k])
                    self.tk_units[(t, k)] = (u, u + n)
                    u += n
                run_nu = u - run_u0
                ni = (run_nu + UMAX - 1) // UMAX
                off = run_u0
                for i in range(ni):
                    sz = (run_nu + ni - 1 - i) // ni  # even split
                    idx = len(self.instrs)
                    self.instrs.append((k, off, sz, g, B))
                    self.tg_instrs[g].append(idx)
                    instr_of_unit.extend([idx] * sz)
                    B += (sz * U + 127) // 128
                    off += sz
                assert off == u
        self.NU = u
        self.NB = B
        self.NI = len(self.instrs)
        self.ntg = ntg
        # pieces per (t, k): list of (instr, batch_in_instr, lo, hi)
        self.tk_pieces = {}
        for (t, k), (u0, u1) in self.tk_units.items():
            pieces = []
            for uu in range(u0, u1):
                i = instr_of_unit[uu]
                slot = (uu - self.instrs[i][1]) * U
                j, lo = slot // 128, slot % 128
                if pieces and pieces[-1][0] == i and pieces[-1][1] == j \
                        and pieces[-1][3] == lo:
                    pieces[-1] = (i, j, pieces[-1][2], lo + U)
                else:
                    pieces.append((i, j, lo, lo + U))
            self.tk_pieces[(t, k)] = pieces


def make_layout(cfg: Cfg, counts):
    """counts: [n_cores, ntiles, n_chunks] edge counts -> static Layout."""
    mx = counts.max(axis=0)
    u = Layout.UNIT
    nb16 = np.maximum((mx + u - 1) // u, 1).astype(np.int64)
    return Layout(cfg, nb16)


# ---------------------------------------------------------------- launch A


def build_launch_a(cfg: Cfg):
    nc = bacc.Bacc(
        "TRN2", target_bir_lowering=False, debug=False, num_devices=cfg.n_cores
    )
    T, K, F, R = cfg.ntiles, cfg.in_dim, cfg.net_dim, cfg.row
    xT_d = nc.dram_tensor("xT", [T, K, 128], mybir.dt.float32, kind="ExternalInput")
    w_d = nc.dram_tensor("w", [K, F], mybir.dt.float32, kind="ExternalInput")
    deg_d = nc.dram_tensor("deg", [128, T], mybir.dt.int32, kind="ExternalInput")
    g_d = nc.dram_tensor("g", [cfg.npad, R], mybir.dt.bfloat16, kind="ExternalOutput")

    with tile.TileContext(nc) as tc:
        with (
            tc.tile_pool(name="const", bufs=1) as cpool,
            tc.tile_pool(name="xin", bufs=4) as xpool,
            tc.tile_pool(name="gout", bufs=4) as gpool,
            tc.tile_pool(name="ph", bufs=4, space="PSUM") as php,
        ):
            w_t = cpool.tile([K, F], mybir.dt.float32)
            nc.sync.dma_start(out=w_t[:], in_=w_d[:])
            dinv_t = _emit_dinv(nc, cpool, deg_d, 128, T, "dinv")

            for t in range(T):
                xT_t = xpool.tile([128, 128], mybir.dt.float32, tag="x")
                nc.sync.dma_start(out=xT_t[:], in_=xT_d[t])
                h_p = php.tile([128, F], mybir.dt.float32, tag="h")
                nc.tensor.matmul(h_p[:], xT_t[:], w_t[:], start=True, stop=True)
                g_t = gpool.tile([128, R], mybir.dt.bfloat16, tag="g")
                nc.scalar.activation(
                    g_t[:, 0:F],
                    h_p[:],
                    mybir.ActivationFunctionType.Copy,
                    scale=dinv_t[:, t : t + 1],
                )
                nc.vector.memset(g_t[:, F:R], 0.0)
                nc.sync.dma_start(out=g_d[t * 128 : (t + 1) * 128, :], in_=g_t[:])
    nc.compile()
    return nc


# ---------------------------------------------------------------- launch B


def build_launch_b(cfg: Cfg, lay: Layout):
    nc = bacc.Bacc(
        "TRN2",
        target_bir_lowering=False,
        debug=False,
        num_devices=cfg.n_cores,
        num_swdge_queues=4,
    )
    F, R, TG = cfg.net_dim, cfg.row, cfg.tg
    NB, NI = lay.NB, lay.NI
    GM = cfg.gmax

    g_d = nc.dram_tensor("g", [cfg.n, R], mybir.dt.bfloat16, kind="ExternalInput")
    src_d = nc.dram_tensor(
        "src_i", [NI, 128, GM * 8], mybir.dt.int16, kind="ExternalInput"
    )
    dstv_d = nc.dram_tensor("dstv", [128, NB], mybir.dt.bfloat16, kind="ExternalInput")
    iota_d = nc.dram_tensor("iota", [128, 128], mybir.dt.bfloat16, kind="ExternalInput")
    degfm_d = nc.dram_tensor("degfm", [F, cfg.npad], mybir.dt.int32, kind="ExternalInput")
    bg_d = nc.dram_tensor("bg", [F, 1], mybir.dt.float32, kind="ExternalInput")
    wd_d = nc.dram_tensor("wd", [F, F], mybir.dt.float32, kind="ExternalInput")
    bd_d = nc.dram_tensor("bd", [F, 1], mybir.dt.float32, kind="ExternalInput")
    out_d = nc.dram_tensor("out", [F, cfg.npad], mybir.dt.float32, kind="ExternalOutput")

    with tile.TileContext(nc) as tc:
        with (
            tc.tile_pool(name="const", bufs=1) as cpool,
            tc.tile_pool(name="idx", bufs=14) as ipool,
            tc.tile_pool(name="msg", bufs=14) as mpool,
            tc.tile_pool(name="oh", bufs=14) as opool,
            tc.tile_pool(name="epi", bufs=2) as epool,
            tc.tile_pool(name="acc", bufs=4, space="PSUM") as apool,
            tc.tile_pool(name="h2", bufs=2, space="PSUM") as hpool,
        ):
            iota_t = cpool.tile([128, 128], mybir.dt.bfloat16)
            nc.sync.dma_start(out=iota_t[:], in_=iota_d[:])
            dstv_t = cpool.tile([128, NB], mybir.dt.bfloat16)
            nc.sync.dma_start(out=dstv_t[:], in_=dstv_d[:])
            bg_t = cpool.tile([F, 1], mybir.dt.float32)
            nc.sync.dma_start(out=bg_t[:], in_=bg_d[:])
            wd_t = cpool.tile([F, F], mybir.dt.float32)
            nc.sync.dma_start(out=wd_t[:], in_=wd_d[:])
            bd_t = cpool.tile([F, 1], mybir.dt.float32)
            nc.sync.dma_start(out=bd_t[:], in_=bd_d[:])
            # dinv feature-major [32, npad], computed in column chunks to
            # bound the temporary SBUF footprint (each temp is npad/4 cols).
            dinv_t = cpool.tile([F, cfg.npad], mybir.dt.float32, name="dinvfm")
            CH = cfg.npad // 4
            with tc.tile_pool(name="dtmp", bufs=1) as dpool:
                for j in range(4):
                    sl = slice(j * CH, (j + 1) * CH)
                    dj = _emit_dinv(nc, dpool, degfm_d[:, sl], F, CH, f"dv{j}")
                    nc.vector.tensor_copy(out=dinv_t[:, sl], in_=dj[:])

            for g in range(lay.ntg):
                tn = lay.tg_sizes[g]
                W = tn * 128
                c0 = g * TG * 128
                # ---- gathers + one-hots for all of this group's instrs
                tiles_of = {}
                for i in lay.tg_instrs[g]:
                    k, u0, nu, _g, bb = lay.instrs[i]
                    nidx = nu * lay.UNIT
                    nbt = (nidx + 127) // 128
                    nw = nidx // 16
                    idx_t = ipool.tile([128, GM * 8], mybir.dt.int16, tag="idx")
                    nc.sync.dma_start(out=idx_t[:, :nw], in_=src_d[i, :, :nw])
                    msg_t = mpool.tile([128, GM, R], mybir.dt.bfloat16, tag="m")
                    nc.gpsimd.dma_gather(
                        msg_t[:, :nbt, :],
                        g_d[k * cfg.chunk : (k + 1) * cfg.chunk, :],
                        idx_t[:, :nw],
                        nidx,
                        nidx,
                        R,
                        queue_num=i % 4,
                    )
                    oh_t = opool.tile([128, GM, 128], mybir.dt.bfloat16, tag="oh")
                    nc.vector.tensor_tensor(
                        out=oh_t[:, :nbt, :],
                        in0=iota_t[:].unsqueeze(1).broadcast_to((128, nbt, 128)),
                        in1=dstv_t[:, bb : bb + nbt]
                        .unsqueeze(2)
                        .broadcast_to((128, nbt, 128)),
                        op=mybir.AluOpType.is_equal,
                    )
                    tiles_of[i] = (msg_t, oh_t)

                # ---- tile-major accumulation: one open PSUM group at a time,
                # each accumulator owns a full bank (start=True resets the
                # bank, so groups must not interleave within one).
                h1_t = epool.tile([F, TG * 128], mybir.dt.float32, tag="h1")
                for tl in range(tn):
                    t = g * TG + tl
                    acc_t = apool.tile([128, 512], mybir.dt.float32, tag="acc")
                    pieces = []
                    for k in range(cfg.n_chunks):
                        pieces.extend(lay.tk_pieces[(t, k)])
                    for j, (i, b, lo, hi) in enumerate(pieces):
                        msg_t, oh_t = tiles_of[i]
                        nc.tensor.matmul(
                            acc_t[0:F, 0:128],
                            msg_t[lo:hi, b, 0:F],
                            oh_t[lo:hi, b, :],
                            start=(j == 0),
                            stop=(j == len(pieces) - 1),
                        )
                    nc.vector.tensor_tensor(
                        out=h1_t[:, tl * 128 : (tl + 1) * 128],
                        in0=acc_t[0:F, 0:128],
                        in1=dinv_t[:, (g * TG + tl) * 128 : (g * TG + tl + 1) * 128],
                        op=mybir.AluOpType.mult,
                    )

                # ---- epilogue for the group
                r1_t = epool.tile([F, TG * 128], mybir.dt.float32, tag="r1")
                nc.scalar.activation(
                    r1_t[:, :W], h1_t[:, :W],
                    mybir.ActivationFunctionType.Relu, bias=bg_t[:],
                )
                h2_p = hpool.tile([F, 512], mybir.dt.float32, tag="h2")
                nc.tensor.matmul(
                    h2_p[:, :W], wd_t[:], r1_t[:, :W], start=True, stop=True
                )
                o_t = epool.tile([F, TG * 128], mybir.dt.float32, tag="o")
                nc.scalar.activation(
                    o_t[:, :W], h2_p[:, :W],
                    mybir.ActivationFunctionType.Relu, bias=bd_t[:],
                )
                nc.sync.dma_start(out=out_d[:, c0 : c0 + W], in_=o_t[:, :W])
    nc.compile()
    return nc


# ---------------------------------------------------------------- host side


def host_prep(x, edge_index, W_gcn, b_gcn, W_dense, b_dense, cfg: Cfg):
    n, nloc = cfg.n, cfg.nloc
    row = np.asarray(edge_index[0]).astype(np.int64)
    col = np.asarray(edge_index[1]).astype(np.int64)
    deg = (np.bincount(col, minlength=n) + 1).astype(np.int32)  # + self-loop

    W_gcn = _f32(W_gcn)
    b_gcn = _f32(b_gcn).reshape(cfg.net_dim, 1)
    W_dense = _f32(W_dense)
    b_dense = _f32(b_dense).reshape(cfg.net_dim, 1)
    x = _f32(x)

    import ml_dtypes

    iota = np.tile(np.arange(128), (128, 1)).astype(ml_dtypes.bfloat16)

    # ---- per-core edge sets (dst-sharded) + self loops
    owner = col // nloc
    per_core = []
    counts = np.zeros((cfg.n_cores, cfg.ntiles, cfg.n_chunks), dtype=np.int64)
    for c in range(cfg.n_cores):
        m = owner == c
        srcs = row[m]
        dstl = col[m] - c * nloc
        loop = np.arange(nloc, dtype=np.int64)
        srcs = np.concatenate([srcs, loop + c * nloc])
        dstl = np.concatenate([dstl, loop])
        t = dstl >> 7
        k = srcs // cfg.chunk
        np.add.at(counts[c], (t, k), 1)
        per_core.append((srcs, dstl, t, k))

    lay = make_layout(cfg, counts)

    in_a, in_b = [], []
    for c in range(cfg.n_cores):
        srcs, dstl, t, k = per_core[c]
        # deg layouts
        dpad = np.ones(cfg.npad, dtype=np.int32)
        dpad[:nloc] = deg[c * nloc : (c + 1) * nloc]
        deg_a = np.ascontiguousarray(dpad.reshape(cfg.ntiles, 128).T)  # [128, T]
        deg_fm = np.ascontiguousarray(
            np.tile(dpad, (cfg.net_dim, 1))
        )  # [32, npad]

        xpad = np.zeros((cfg.npad, cfg.in_dim), dtype=np.float32)
        xpad[:nloc] = x[c * nloc : (c + 1) * nloc]
        xT3 = np.ascontiguousarray(
            xpad.reshape(cfg.ntiles, 128, cfg.in_dim).transpose(0, 2, 1)
        )
        in_a.append({"xT": xT3, "w": W_gcn, "deg": deg_a})

        # ---- slot assignment: unit stream in (TG, k, t) layout order
        U = lay.UNIT
        src_slots = np.zeros(lay.NU * U, dtype=np.int64)
        dst_slots = np.full(lay.NU * U, -1.0, dtype=np.float64)
        base = np.zeros((cfg.ntiles, cfg.n_chunks), dtype=np.int64)
        for (tt, kk), (u0, _u1) in lay.tk_units.items():
            base[tt, kk] = u0 * U
        order = np.lexsort((k, t))
        ts, ks = t[order], k[order]
        so = (srcs[order] - ks * cfg.chunk).astype(np.int64)
        do = (dstl[order] & 127).astype(np.int64)
        grp = ts * cfg.n_chunks + ks
        uq, starts_, cnts_ = np.unique(grp, return_index=True, return_counts=True)
        rank = np.arange(grp.size) - np.repeat(starts_, cnts_)
        slot = base[ts, ks] + rank
        src_slots[slot] = so
        dst_slots[slot] = do

        src_i = np.zeros((lay.NI, 128, cfg.gmax * 8), dtype=np.int16)
        dstv = np.full((128, lay.NB), -1.0, dtype=np.float64)
        for i, (kk, u0, nu, _g, bb) in enumerate(lay.instrs):
            seg = src_slots[u0 * U : (u0 + nu) * U]
            src_i[i, :, : nu * U // 16] = wrap16(seg)
            dseg = dst_slots[u0 * U : (u0 + nu) * U]
            nbt = (nu * U + 127) // 128
            pad = np.full(nbt * 128, -1.0)
            pad[: nu * U] = dseg
            dstv[:, bb : bb + nbt] = pad.reshape(nbt, 128).T
        dstv = np.ascontiguousarray(dstv).astype(ml_dtypes.bfloat16)

        in_b.append(
            {
                "src_i": src_i,
                "dstv": dstv,
                "iota": iota,
                "degfm": deg_fm,
                "bg": b_gcn,
                "wd": W_dense,
                "bd": b_dense,
            }
        )
    return in_a, in_b, lay


def assemble_table(res_a, cfg: Cfg):
    return np.ascontiguousarray(
        np.concatenate([res_a[c]["g"][: cfg.nloc] for c in range(cfg.n_cores)], axis=0)
    )


def assemble_out(res_b, cfg: Cfg):
    return np.ascontiguousarray(
        np.concatenate(
            [res_b[c]["out"][:, : cfg.nloc].T for c in range(cfg.n_cores)], axis=0
        )
    ).astype(np.float32)


def _add_table(in_b, table, cfg: Cfg):
    for m in in_b:
        m["g"] = table


def kernel(x, edge_index, W_gcn, b_gcn, W_dense, b_dense):
    cfg = FULL
    in_a, in_b, lay = host_prep(x, edge_index, W_gcn, b_gcn, W_dense, b_dense, cfg)
    nc_a = build_launch_a(cfg)
    nc_b = build_launch_b(cfg, lay)
    core_ids = list(range(cfg.n_cores))
    res_a = run_bass_kernel_spmd(nc_a, in_a, core_ids).results
    table = assemble_table(res_a, cfg)
    _add_table(in_b, table, cfg)
    res_b = run_bass_kernel_spmd(nc_b, in_b, core_ids).results
    return assemble_out(res_b, cfg)
